# revision 18
# baseline (speedup 1.0000x reference)
"""Trainium2 Bass kernel: fused multi-head attention (N=4, L=2048, E=2048, H=16).

Sharding (8 cores): data-parallel over the 4 batches x tensor-parallel over 2
head-groups of 8 heads.  Core c handles batch c//2, head-group c%2.  Each core
computes Q/K/V projections for its head group, masked softmax attention, and
the partial output projection against its row-slice of Wo.  The two partials
per batch are summed on the host (the Wo row-parallel all-reduce) and the
output bias is added there too.

Per-core kernel layout choices (all matmuls contraction-on-partitions, no
on-device transposes -- the host ships pre-transposed activations/weights):
  - Q^T, K^T per head: [d=128, L] via lhsT=W^T chunk, rhs=X^T chunk
  - scores S^T tile: [k-block=128, L] = (K^T chunk).T @ Q^T
  - P = exp(S^T * E^-0.5 + mask_bias[k]) directly on ScalarE (mask folded
    into the per-partition activation bias), output bf16
  - out^T_h accumulated in PSUM via lhsT=V chunk, rhs=P chunk
  - softmax denominator: DVE-accumulate P chunks, then a ones-matmul both
    partition-reduces and broadcasts it; reciprocal + multiply normalizes
  - output projection: lhsT=A^T blocks, rhs=Wo^T, accumulated over heads
"""

import os
from contextlib import ExitStack

import numpy as np
import ml_dtypes

P = 128          # SBUF partitions
D = 128          # head dim
G = 2            # head groups (tensor-parallel degree per batch)
NCORES = 8
BF16 = ml_dtypes.bfloat16
MASK_BIAS = -60.0

_BUILT = {}
LAST_EXEC_NS = None


def _build(L, E, HL, reps=1):
    """Build the per-core Bass module (same program on every core).

    reps>1 repeats the whole computation serially inside one NEFF (scratch
    WAW deps order the reps) -- used only for slope-based HW timing."""
    import concourse.bass as bass
    import concourse.tile as tile
    from concourse import mybir

    bf16 = mybir.dt.bfloat16
    f32 = mybir.dt.float32
    f32r = mybir.dt.float32r

    EH = HL * D          # local projection width
    IC = E // P          # contraction chunks (projections)
    ICH = IC // 2        # half of the contraction chunks (split W/X loads)
    KC = L // P          # key chunks (attention)
    NT = L // 512        # 512-wide l tiles
    ET = E // 512        # 512-wide e tiles (out proj)
    VW = min(512, EH // 2)  # v-proj dh tile width (within one W half-load)
    VT = EH // VW
    KB = 512 // P        # k blocks per 512-wide l tile (v proj)
    LH = min(1024, L)    # attention l-half width
    NLH = L // LH
    NTH = LH // 512
    SCALE = float(E) ** -0.5

    nc = bass.Bass(num_swdge_queues=4)
    xqt = nc.dram_tensor("xqt", [E, L], bf16, kind="ExternalInput")
    xkt = nc.dram_tensor("xkt", [E, L], bf16, kind="ExternalInput")
    xvt = nc.dram_tensor("xvt", [E, L], bf16, kind="ExternalInput")
    wqt = nc.dram_tensor("wqt", [E, EH], bf16, kind="ExternalInput")
    wkt = nc.dram_tensor("wkt", [E, EH], bf16, kind="ExternalInput")
    wvt = nc.dram_tensor("wvt", [E, EH], bf16, kind="ExternalInput")
    wot = nc.dram_tensor("wot", [EH, E], bf16, kind="ExternalInput")
    mbias = nc.dram_tensor("mbias", [P, KC], f32, kind="ExternalInput")
    out = nc.dram_tensor("out", [L, E], f32, kind="ExternalOutput")

    kt_d = nc.dram_tensor("kt_scratch", [HL, P, L], bf16)
    v_d = nc.dram_tensor("v_scratch", [L, EH], bf16)

    with tile.TileContext(nc) as tc, ExitStack() as ctx:
        # All SBUF pools are opened for the whole kernel so no SBUF address is
        # ever reused across pools (cross-pool aliasing generates WAR waits
        # against many DMA-queue processors -> "too many sync wait commands").
        singles = ctx.enter_context(tc.tile_pool(name="singles", bufs=1))
        at_pool = ctx.enter_context(tc.tile_pool(name="at", bufs=1))
        xp = ctx.enter_context(tc.tile_pool(name="xp", bufs=2))
        wp = ctx.enter_context(tc.tile_pool(name="wp", bufs=2))
        op_ = ctx.enter_context(tc.tile_pool(name="op", bufs=4))
        kqp = ctx.enter_context(tc.tile_pool(name="kq", bufs=2))
        qtp = ctx.enter_context(tc.tile_pool(name="qt", bufs=2))
        vp = ctx.enter_context(tc.tile_pool(name="vpool", bufs=2))
        ptp = ctx.enter_context(tc.tile_pool(name="ptp", bufs=3))
        dnp = ctx.enter_context(tc.tile_pool(name="dnp", bufs=2))

        ones_f32 = singles.tile([P, P], f32)
        nc.vector.memset(ones_f32, 1.0)
        ones_t = singles.tile([P, P], f32r)
        nc.vector.tensor_copy(out=ones_t, in_=ones_f32)
        mb_t = singles.tile([P, KC], f32)
        nc.sync.dma_start(out=mb_t, in_=mbias[:, :])

        at_t = at_pool.tile([P, HL, L], bf16)

        # ---- K / V projections (spilled to DRAM scratch) ----
        EHH = EH // 2  # W loaded in two dh-halves so loads pipeline (bufs=2)

        def proj(xT, wT, name, transposed_out):  # noqa: C901
            xv = xT.rearrange("(c p) l -> p c l", p=P)
            wv = wT.rearrange("(c p) m -> p c m", p=P)
            with tc.tile_pool(name=f"ps_{name}", bufs=4, space="PSUM") as pp:
                wts = []
                for wh in range(2):
                    wt = wp.tile([P, IC, EHH], bf16, tag="w")
                    for c in range(IC):
                        nc.sync.dma_start(
                            out=wt[:, c],
                            in_=wv[:, c, wh * EHH:(wh + 1) * EHH])
                    wts.append(wt)

                def w_slice(j0, j1):  # dh range -> (tile, local slice)
                    wh = j0 // EHH
                    assert (j1 - 1) // EHH == wh
                    return wts[wh][:, :, j0 - wh * EHH:j1 - wh * EHH]

                for lt in range(NT):
                    xt = xp.tile([P, IC, 512], bf16, tag="x")
                    for c in range(IC):
                        nc.sync.dma_start(
                            out=xt[:, c], in_=xv[:, c, lt * 512:(lt + 1) * 512])
                    if transposed_out is not None:  # K^T: [d, l] per head
                        for h in range(HL):
                            ps = pp.tile([P, 512], f32, tag="ps")
                            wsl = w_slice(h * D, (h + 1) * D)
                            for c in range(IC):
                                nc.tensor.matmul(
                                    ps, lhsT=wsl[:, c],
                                    rhs=xt[:, c],
                                    start=(c == 0), stop=(c == IC - 1))
                            ot = op_.tile([P, 512], bf16, tag="o")
                            nc.vector.tensor_copy(out=ot, in_=ps)
                            nc.sync.dma_start(
                                out=transposed_out[h, :, lt * 512:(lt + 1) * 512],
                                in_=ot)
                    else:  # V: natural [k, dh]
                        for kb in range(KB):
                            for vt_ in range(VT):
                                ps = pp.tile([P, VW], f32, tag="ps")
                                wsl = w_slice(vt_ * VW, (vt_ + 1) * VW)
                                for c in range(IC):
                                    nc.tensor.matmul(
                                        ps, lhsT=xt[:, c, kb * P:(kb + 1) * P],
                                        rhs=wsl[:, c],
                                        start=(c == 0), stop=(c == IC - 1))
                                ot = op_.tile([P, VW], bf16, tag="o")
                                nc.vector.tensor_copy(out=ot, in_=ps)
                                r0 = lt * 512 + kb * P
                                nc.sync.dma_start(
                                    out=v_d[r0:r0 + P, vt_ * VW:(vt_ + 1) * VW],
                                    in_=ot)

        def body(rep):
          proj(xkt, wkt, f"k{rep}", kt_d)
          proj(xvt, wvt, f"v{rep}", None)
          run_attention(rep)
          run_outproj(rep)

        # ---- fused Q-projection + attention ----
        # Computing Q^T per (head, l-half) right before its attention keeps
        # TensorE dense through the ScalarE-heavy softmax phase (HAM stays
        # warm) and avoids spilling Q^T to DRAM.
        xq_v = xqt.rearrange("(c p) l -> p c l", p=P)
        wq_v = wqt.rearrange("(c p) m -> p c m", p=P)
        v_view = v_d.rearrange("(c p) m -> p c m", p=P)

        def run_attention(rep):
          wq_halves = []
          for wh in range(2):  # ic-halves
              wqh = wp.tile([P, ICH, EH], bf16, tag="w")
              for c in range(ICH):
                  nc.sync.dma_start(out=wqh[:, c], in_=wq_v[:, wh * ICH + c])
              wq_halves.append(wqh)

          with tc.tile_pool(name=f"stps{rep}", bufs=2, space="PSUM") as stp, \
               tc.tile_pool(name=f"otps{rep}", bufs=1, space="PSUM") as otp, \
               tc.tile_pool(name=f"qps{rep}", bufs=1, space="PSUM") as qpp:
            for lh in range(NLH):
                l0 = lh * LH
                xq_halves = []
                for wh in range(2):  # ic-halves of this l-half of X_q^T
                    xqh = xp.tile([P, ICH, LH], bf16, tag="x2")
                    for c in range(ICH):
                        nc.sync.dma_start(
                            out=xqh[:, c],
                            in_=xq_v[:, wh * ICH + c, l0:l0 + LH])
                    xq_halves.append(xqh)
                for h in range(HL):
                    kt_t = kqp.tile([P, L], bf16, tag="kt")
                    nc.sync.dma_start(out=kt_t, in_=kt_d[h])
                    v_t = vp.tile([P, KC, D], bf16, tag="v")
                    nc.sync.dma_start(
                        out=v_t, in_=v_view[:, :, h * D:(h + 1) * D])
                    # Q^T slice for this (head, l-half)
                    q_ps = qpp.tile([P, LH], f32, tag="q")
                    for nt in range(NTH):
                        for c in range(IC):
                            nc.tensor.matmul(
                                q_ps[:, nt * 512:(nt + 1) * 512],
                                lhsT=wq_halves[c // ICH][
                                    :, c % ICH, h * D:(h + 1) * D],
                                rhs=xq_halves[c // ICH][
                                    :, c % ICH, nt * 512:(nt + 1) * 512],
                                start=(c == 0), stop=(c == IC - 1))
                    qt_t = qtp.tile([P, LH], bf16, tag="qt")
                    nc.vector.tensor_copy(out=qt_t, in_=q_ps)

                    ot_ps = otp.tile([P, LH], f32, tag="ot")
                    den = dnp.tile([P, LH], f32r, tag="den")
                    for c in range(KC):
                        st = stp.tile([P, LH], f32, tag="st")
                        for nt in range(NTH):
                            nc.tensor.matmul(
                                st[:, nt * 512:(nt + 1) * 512],
                                lhsT=kt_t[:, c * P:(c + 1) * P],
                                rhs=qt_t[:, nt * 512:(nt + 1) * 512],
                                start=True, stop=True)
                        pt = ptp.tile([P, LH], bf16, tag="pt")
                        nc.scalar.activation(
                            out=pt, in_=st,
                            func=mybir.ActivationFunctionType.Exp,
                            bias=mb_t[:, c:c + 1], scale=SCALE)
                        if c == 0:
                            nc.vector.tensor_copy(out=den, in_=pt)
                        else:
                            nc.vector.tensor_add(out=den, in0=den, in1=pt)
                        for nt in range(NTH):
                            nc.tensor.matmul(
                                ot_ps[:, nt * 512:(nt + 1) * 512],
                                lhsT=v_t[:, c],
                                rhs=pt[:, nt * 512:(nt + 1) * 512],
                                start=(c == 0), stop=(c == KC - 1))
                    # denominator: partition-reduce + broadcast via ones-matmul
                    bc = stp.tile([P, LH], f32, tag="st")
                    for nt in range(NTH):
                        nc.tensor.matmul(
                            bc[:, nt * 512:(nt + 1) * 512],
                            lhsT=ones_t[:, :],
                            rhs=den[:, nt * 512:(nt + 1) * 512],
                            start=True, stop=True)
                    rec = dnp.tile([P, LH], f32, tag="den")
                    nc.vector.reciprocal(out=rec, in_=bc)
                    nc.vector.tensor_mul(out=at_t[:, h, l0:l0 + LH],
                                         in0=ot_ps, in1=rec)

        # ---- output projection ----
        wot_v = wot.rearrange("(h p) e -> p h e", p=P)
        HLH = max(1, HL // 2)

        def run_outproj(rep):
          # Wo^T loads into the weight pool slots freed after the Q weights.
          wo_halves = []
          for wh in range(2 if HL > 1 else 1):
              woh = wp.tile([P, HLH, E], bf16, tag="w")
              for j in range(HLH):
                  nc.sync.dma_start(out=woh[:, j], in_=wot_v[:, wh * HLH + j])
              wo_halves.append(woh)

          with tc.tile_pool(name=f"oo{rep}", bufs=4) as oop, \
               tc.tile_pool(name=f"ops{rep}", bufs=4, space="PSUM") as opp:
            for lb in range(L // P):
                for et in range(ET):
                    ps = opp.tile([P, 512], f32, tag="ps")
                    for h in range(HL):
                        nc.tensor.matmul(
                            ps, lhsT=at_t[:, h, lb * P:(lb + 1) * P],
                            rhs=wo_halves[h // HLH][
                                :, h % HLH, et * 512:(et + 1) * 512],
                            start=(h == 0), stop=(h == HL - 1))
                    ot = oop.tile([P, 512], f32, tag="o")
                    nc.vector.tensor_copy(out=ot, in_=ps)
                    nc.sync.dma_start(
                        out=out[lb * P:(lb + 1) * P, et * 512:(et + 1) * 512],
                        in_=ot)

        for rep in range(reps):
            body(rep)

    # Split multi-wait sync_infos (TRN2 instructions carry at most one wait;
    # only the Bacc path runs this pass by default).
    import bass_rust
    bass_rust.move_matmul_waits_to_ldweights(nc.m)
    bass_rust.generate_event_semaphores(nc)
    return nc


def _get_nc(L, E, HL):
    key = (L, E, HL)
    if key not in _BUILT:
        _BUILT[key] = _build(L, E, HL)
    return _BUILT[key]


def _core_inputs(query_n, keys_n, values_n, mask_n, Wq, Wk, Wv, Wo, g, L, E, HL):
    """Host-side shard prep for one core: transpose + bf16-cast the batch's
    activations and the head-group's weight slices."""
    EH = HL * D
    sl = slice(g * EH, (g + 1) * EH)
    # additive exp bias: 0 for kept keys, MASK_BIAS for masked ones
    mb = (1.0 - mask_n.astype(np.float32)) * MASK_BIAS
    mb = np.ascontiguousarray(mb.reshape(L // P, P).T)     # [P, KC]
    return {
        "xqt": query_n.T.astype(BF16, order="C"),
        "xkt": keys_n.T.astype(BF16, order="C"),
        "xvt": values_n.T.astype(BF16, order="C"),
        "wqt": Wq[sl, :].T.astype(BF16, order="C"),
        "wkt": Wk[sl, :].T.astype(BF16, order="C"),
        "wvt": Wv[sl, :].T.astype(BF16, order="C"),
        "wot": Wo[:, sl].T.astype(BF16, order="C"),
        "mbias": mb.astype(np.float32),
    }


def kernel(query, keys, values, mask, Wq, Wk, Wv, Wo, bo):
    global LAST_EXEC_NS
    from concourse.bass_utils import run_bass_kernel_spmd

    query = np.asarray(query, dtype=np.float32)
    keys = np.asarray(keys, dtype=np.float32)
    values = np.asarray(values, dtype=np.float32)
    mask = np.asarray(mask)
    Wq = np.asarray(Wq, dtype=np.float32)
    Wk = np.asarray(Wk, dtype=np.float32)
    Wv = np.asarray(Wv, dtype=np.float32)
    Wo = np.asarray(Wo, dtype=np.float32)
    bo = np.asarray(bo, dtype=np.float32)

    N, L, E = query.shape
    H = 16
    HL = H // G

    nc = _get_nc(L, E, HL)

    in_maps = []
    for c in range(NCORES):
        n, g = divmod(c, G)
        in_maps.append(_core_inputs(
            query[n], keys[n], values[n], mask[n], Wq, Wk, Wv, Wo, g, L, E, HL))

    res = run_bass_kernel_spmd(nc, in_maps, core_ids=list(range(NCORES)))
    LAST_EXEC_NS = res.exec_time_ns

    out = np.empty((N, L, E), np.float32)
    for n in range(N):
        acc = res.results[G * n]["out"].copy()
        for g in range(1, G):
            acc += res.results[G * n + g]["out"]
        out[n] = acc + bo[None, :]
    return out
